# revision 3
# baseline (speedup 1.0000x reference)
"""Trainium2 Bass kernel for MQA attention (B=2, T=4096, D=896, H=7, HD=128).

Distribution: 8 cores = 2 batches x 4 row-groups. Core (b,p) computes query
rows g = 4*m + p of batch b (strided assignment keeps every core's program
identical; causal loop bounds are uniform and the causal boundary enters only
through a per-core data mask).

Per core:
  - kT = rope(Wk^T @ xT), vT = Wv^T @ xT (full batch, replicated across the
    4 cores of the batch; MQA so K/V projections are small), v = transpose(vT)
  - qT_h = rope(Wq_h^T @ xq) for the core's 1024 strided query rows
  - scoresT = kT_chunk^T @ qT (contraction over HD on the PE), exp on the
    scalar engine (no max subtraction: logits are O(1) by construction),
    causal mask as a 0/1 multiply on the 32-column diagonal boundary region,
    av^T accumulated on the PE with v chunks as stationary, denominators via
    a ones-vector matmul, normalization after the k-loop.
  - out rows = sum_h (av_h/den_h)^T @ Wo_h

RoPE uses a host-side column permutation of Wq/Wk so the rotation partner of
partition p lives 16 partitions away within the same 32-partition quadrant
(reachable by one DVE stream_shuffle); cos/sin tables are host-expanded to
[128, T] with signs folded in.

All matmul inputs are bf16 (fp32 accumulation); verified ~0.5% rel error.
"""

import math

import ml_dtypes
import numpy as np

import concourse.bass as bass
import concourse.mybir as mybir
import concourse.tile as tile
from concourse import bacc
from concourse.bass_utils import run_bass_kernel_spmd

B, T, D = 2, 4096, 896
H, HD = 7, 128
DC = D // 128  # 7 contraction chunks
NCORES = 8
QLOC = 1024  # local query rows per core
QW = 512  # query window width
NW = QLOC // QW  # 2
NKT = T // 128  # 32
MW = 32  # causal boundary mask width
SCALE = 1.0 / math.sqrt(HD)

BF = mybir.dt.bfloat16
F32 = mybir.dt.float32
npbf = ml_dtypes.bfloat16

SHUF = [(i + 16) % 32 for i in range(32)]
EXP = mybir.ActivationFunctionType.Exp


def _emit(nc):
    xt = nc.dram_tensor("xt", [D, T], BF, kind="ExternalInput").ap()
    xq = nc.dram_tensor("xq", [D, QLOC], BF, kind="ExternalInput").ap()
    wq = nc.dram_tensor("wq", [D, H * HD], BF, kind="ExternalInput").ap()
    wk = nc.dram_tensor("wk", [D, HD], BF, kind="ExternalInput").ap()
    wv = nc.dram_tensor("wv", [D, HD], BF, kind="ExternalInput").ap()
    wo = nc.dram_tensor("wo", [H * HD, D], BF, kind="ExternalInput").ap()
    cq = nc.dram_tensor("cq", [HD, QLOC], F32, kind="ExternalInput").ap()
    sq = nc.dram_tensor("sq", [HD, QLOC], F32, kind="ExternalInput").ap()
    ck = nc.dram_tensor("ck", [HD, T], F32, kind="ExternalInput").ap()
    sk = nc.dram_tensor("sk", [HD, T], F32, kind="ExternalInput").ap()
    msk = nc.dram_tensor("msk", [HD, MW], BF, kind="ExternalInput").ap()
    out = nc.dram_tensor("out", [QLOC, D], F32, kind="ExternalOutput").ap()

    with tile.TileContext(nc) as tc:
        with (
            tc.tile_pool(name="const", bufs=1) as const,
            tc.tile_pool(name="persist", bufs=1) as persist,
        ):
            # ---------- constants ----------
            wq_sb = const.tile([128, DC, H * HD], BF)
            nc.sync.dma_start(out=wq_sb, in_=wq.rearrange("(dc p) n -> p dc n", p=128))
            wk_sb = const.tile([128, DC, HD], BF)
            nc.sync.dma_start(out=wk_sb, in_=wk.rearrange("(dc p) n -> p dc n", p=128))
            wv_sb = const.tile([128, DC, HD], BF)
            nc.sync.dma_start(out=wv_sb, in_=wv.rearrange("(dc p) n -> p dc n", p=128))
            wo_sb = const.tile([128, H, D], BF)
            nc.sync.dma_start(out=wo_sb, in_=wo.rearrange("(h p) n -> p h n", p=128))
            cq_sb = const.tile([128, QLOC], F32)
            nc.sync.dma_start(out=cq_sb, in_=cq)
            sq_sb = const.tile([128, QLOC], F32)
            nc.sync.dma_start(out=sq_sb, in_=sq)
            ck_sb = const.tile([128, T], F32)
            nc.sync.dma_start(out=ck_sb, in_=ck)
            sk_sb = const.tile([128, T], F32)
            nc.sync.dma_start(out=sk_sb, in_=sk)
            msk_sb = const.tile([128, MW], BF)
            nc.sync.dma_start(out=msk_sb, in_=msk)
            ones_sb = const.tile([128, 1], BF)
            nc.vector.memset(ones_sb, 1.0)

            # ---------- persistent intermediates ----------
            kr_sb = persist.tile([128, T], BF)  # rope'd kT
            v_sb = persist.tile([128, NKT, 128], BF)  # v natural, chunked
            qr_sb = persist.tile([128, H, QLOC], BF)  # rope'd qT per head

            # ---------- phase B: projections + rope ----------
            with (
                tc.tile_pool(name="xpool", bufs=1) as xpool,
                tc.tile_pool(name="rt", bufs=3) as rt,
                tc.tile_pool(name="pb", bufs=4, space="PSUM") as pb,
            ):
                xt_sb = xpool.tile([128, DC, T], BF)
                nc.sync.dma_start(
                    out=xt_sb, in_=xt.rearrange("(dc p) t -> p dc t", p=128)
                )
                xq_sb = xpool.tile([128, DC, QLOC], BF)
                nc.sync.dma_start(
                    out=xq_sb, in_=xq.rearrange("(dc p) t -> p dc t", p=128)
                )
                vt_sb = xpool.tile([128, T], BF)  # vT staging

                def rope(ps, cos_ap, sin_ap, out_ap):
                    qs = rt.tile([128, QW], F32, tag="qs", name="qs")
                    nc.vector.stream_shuffle(qs, ps, SHUF)
                    t1 = rt.tile([128, QW], F32, tag="t1", name="t1")
                    nc.vector.tensor_mul(t1, ps, cos_ap)
                    t2 = rt.tile([128, QW], F32, tag="t2", name="t2")
                    nc.vector.tensor_mul(t2, qs, sin_ap)
                    nc.vector.tensor_add(out_ap, t1, t2)

                # k projection + rope (full T)
                for w in range(T // QW):
                    sl = bass.ts(w, QW)
                    ps = pb.tile([128, QW], F32, tag="ps", name="ps")
                    for dc in range(DC):
                        nc.tensor.matmul(
                            ps,
                            lhsT=wk_sb[:, dc, :],
                            rhs=xt_sb[:, dc, sl],
                            start=dc == 0,
                            stop=dc == DC - 1,
                        )
                    rope(ps, ck_sb[:, sl], sk_sb[:, sl], kr_sb[:, sl])

                # v projection (as vT), then DMA-transpose into natural layout
                for w in range(T // QW):
                    sl = bass.ts(w, QW)
                    ps = pb.tile([128, QW], F32, tag="ps", name="ps")
                    for dc in range(DC):
                        nc.tensor.matmul(
                            ps,
                            lhsT=wv_sb[:, dc, :],
                            rhs=xt_sb[:, dc, sl],
                            start=dc == 0,
                            stop=dc == DC - 1,
                        )
                    nc.vector.tensor_copy(vt_sb[:, sl], ps)
                for tt in range(NKT):
                    nc.sync.dma_start_transpose(
                        out=v_sb[:, tt, :], in_=vt_sb[:, bass.ts(tt, 128)]
                    )

                # q projection + rope, per head
                for h in range(H):
                    for w in range(NW):
                        sl = bass.ts(w, QW)
                        ps = pb.tile([128, QW], F32, tag="ps", name="ps")
                        for dc in range(DC):
                            nc.tensor.matmul(
                                ps,
                                lhsT=wq_sb[:, dc, bass.ts(h, HD)],
                                rhs=xq_sb[:, dc, sl],
                                start=dc == 0,
                                stop=dc == DC - 1,
                            )
                        rope(ps, cq_sb[:, sl], sq_sb[:, sl], qr_sb[:, h, sl])

            # ---------- phase C: attention ----------
            with (
                tc.tile_pool(name="avpool", bufs=1) as avpool,
                tc.tile_pool(name="expool", bufs=4) as expool,
                tc.tile_pool(name="nrm", bufs=2) as nrm,
                tc.tile_pool(name="outp", bufs=2) as outp,
                tc.tile_pool(name="ps_s", bufs=3, space="PSUM") as ps_s,
                tc.tile_pool(name="ps_av", bufs=2, space="PSUM") as ps_av,
                tc.tile_pool(name="ps_den", bufs=1, space="PSUM") as ps_den,
                tc.tile_pool(name="ps_o", bufs=2, space="PSUM") as ps_o,
            ):
                av_sb = avpool.tile([128, H, QLOC], BF)

                for h in range(H):
                    for w in range(NW):
                        n_k = 16 * (w + 1)
                        nd = n_k - 16  # full (non-diagonal) chunks
                        av_ps = ps_av.tile([128, QW], F32, tag="av", name="av")
                        den_ps = ps_den.tile([1, QW], F32, tag="den", name="den")

                        pend = None  # software pipeline: (ex, j0, kb)
                        for kb in range(n_k):
                            c = kb - nd
                            j0 = 0 if c < 0 else 32 * c
                            s_ps = ps_s.tile([128, QW], F32, tag="s", name="s")
                            nc.tensor.matmul(
                                s_ps[:, j0:],
                                lhsT=kr_sb[:, bass.ts(kb, 128)],
                                rhs=qr_sb[:, h, w * QW + j0 : (w + 1) * QW],
                                start=True,
                                stop=True,
                            )
                            ex = expool.tile([128, QW], BF, tag="ex", name="ex")
                            nc.scalar.activation(
                                ex[:, j0:], s_ps[:, j0:], func=EXP, scale=SCALE
                            )
                            if c >= 0:
                                nc.vector.tensor_mul(
                                    ex[:, j0 : j0 + MW], ex[:, j0 : j0 + MW], msk_sb
                                )
                            if pend is not None:
                                pex, pj0, pkb = pend
                                nc.tensor.matmul(
                                    av_ps[:, pj0:],
                                    lhsT=v_sb[:, pkb, :],
                                    rhs=pex[:, pj0:],
                                    start=pkb == 0,
                                    stop=False,
                                )
                                nc.tensor.matmul(
                                    den_ps[:, pj0:],
                                    lhsT=ones_sb,
                                    rhs=pex[:, pj0:],
                                    start=pkb == 0,
                                    stop=False,
                                )
                            pend = (ex, j0, kb)
                        pex, pj0, pkb = pend
                        nc.tensor.matmul(
                            av_ps[:, pj0:],
                            lhsT=v_sb[:, pkb, :],
                            rhs=pex[:, pj0:],
                            start=pkb == 0,
                            stop=True,
                        )
                        nc.tensor.matmul(
                            den_ps[:, pj0:],
                            lhsT=ones_sb,
                            rhs=pex[:, pj0:],
                            start=pkb == 0,
                            stop=True,
                        )

                        # normalize: av_sb[:, h, w] = av / den
                        den_sb = nrm.tile([1, QW], F32, tag="den_sb", name="den_sb")
                        nc.scalar.copy(den_sb, den_ps)
                        rec = nrm.tile([1, QW], F32, tag="rec", name="rec")
                        nc.vector.reciprocal(rec, den_sb)
                        db = nrm.tile([128, QW], F32, tag="db", name="db")
                        nc.gpsimd.partition_broadcast(db, rec)
                        nc.vector.tensor_mul(
                            av_sb[:, h, bass.ts(w, QW)], av_ps, db
                        )

                # ---------- phase D: output projection ----------
                NO = 448  # half of D, fits one PSUM bank
                for s in range(QLOC // 128):
                    ob = outp.tile([128, D], F32, tag="ob", name="ob")
                    for half in range(2):
                        o_ps = ps_o.tile([128, NO], F32, tag="o", name="o")
                        for h in range(H):
                            nc.tensor.matmul(
                                o_ps,
                                lhsT=av_sb[:, h, bass.ts(s, 128)],
                                rhs=wo_sb[:, h, bass.ts(half, NO)],
                                start=h == 0,
                                stop=h == H - 1,
                            )
                        nc.vector.tensor_copy(ob[:, bass.ts(half, NO)], o_ps)
                    nc.sync.dma_start(out=out[bass.ts(s, 128), :], in_=ob)


_CACHE = {}


def _get_nc():
    if "nc" not in _CACHE:
        nc = bacc.Bacc("TRN2", target_bir_lowering=False, debug=False)
        _emit(nc)
        nc.compile()
        _CACHE["nc"] = nc
    return _CACHE["nc"]


def _rope_layout():
    p = np.arange(128)
    q, r = p // 32, p % 32
    j = 16 * q + (r % 16)
    is_b = r >= 16
    pi = 2 * j + is_b.astype(int)
    sign = np.where(is_b, 1.0, -1.0).astype(np.float32)
    return pi, j, sign


def _prepare_in_maps(x, Wq, Wk, Wv, Wo, cos, sin):
    x = np.asarray(x, dtype=np.float32)
    Wq = np.asarray(Wq, dtype=np.float32)
    Wk = np.asarray(Wk, dtype=np.float32)
    Wv = np.asarray(Wv, dtype=np.float32)
    Wo = np.asarray(Wo, dtype=np.float32)
    cos = np.asarray(cos, dtype=np.float32)
    sin = np.asarray(sin, dtype=np.float32)

    pi, j_of_p, sign = _rope_layout()
    wq_p = np.ascontiguousarray(
        Wq.reshape(D, H, HD)[:, :, pi].reshape(D, H * HD)
    ).astype(npbf)
    wk_p = np.ascontiguousarray(Wk[:, pi]).astype(npbf)
    wv_b = Wv.astype(npbf)
    wo_b = Wo.astype(npbf)
    cosE = np.ascontiguousarray(cos.T[j_of_p, :])  # [128, T] f32
    sinE = np.ascontiguousarray(cos.T[j_of_p, :] * 0)  # placeholder, set below
    sinE = np.ascontiguousarray(sin.T[j_of_p, :] * sign[:, None])

    in_maps = []
    for c_id in range(NCORES):
        b, p = divmod(c_id, 4)
        rows = 4 * np.arange(QLOC) + p
        xt_b = np.ascontiguousarray(x[b].T).astype(npbf)  # [D, T]
        xq_b = np.ascontiguousarray(xt_b[:, rows])
        msk = (
            np.arange(128)[:, None] <= 4 * np.arange(MW)[None, :] + p
        ).astype(npbf)
        in_maps.append(
            {
                "xt": xt_b,
                "xq": xq_b,
                "wq": wq_p,
                "wk": wk_p,
                "wv": wv_b,
                "wo": wo_b,
                "cq": np.ascontiguousarray(cosE[:, rows]),
                "sq": np.ascontiguousarray(sinE[:, rows]),
                "ck": cosE,
                "sk": sinE,
                "msk": msk,
            }
        )
    return in_maps


def kernel(x, Wq, Wk, Wv, Wo, cos, sin):
    in_maps = _prepare_in_maps(x, Wq, Wk, Wv, Wo, cos, sin)
    nc = _get_nc()
    res = run_bass_kernel_spmd(nc, in_maps, core_ids=list(range(NCORES)))
    out_full = np.zeros((B, T, D), dtype=np.float32)
    for c_id in range(NCORES):
        b, p = divmod(c_id, 4)
        out_full[b, p::4, :] = res.results[c_id]["out"]
    return out_full


# revision 12
# speedup vs baseline: 51.5178x; 51.5178x over previous
"""Trainium2 Bass kernel for MQA attention (B=2, T=4096, D=896, H=7, HD=128).

Distribution: 8 cores = 2 batches x 4 row-groups. Core (b,p) computes query
rows g = 4*m + p of batch b (strided assignment keeps every core's program
identical; the causal boundary enters only through a per-core data mask).

Per core:
  - kT = rope(Wk^T @ xT), vT = Wv^T @ xT (full batch; MQA so K/V projections
    are small and replicated across the 4 cores of a batch), v = T(vT) via
    DMA transpose.
  - per head pair (h0,h1): qT = rope(Wq_h^T @ xq); attention with scoresT
    layout: scoresT = kT_chunk^T @ qT (PE, k-chunk stationary shared by the
    head pair), exp on ScalarE over both heads in one ACTIVATE (halves the
    352-cycle pipe-fill cost), causal 0/1 mask multiply on the 32-column
    boundary region, av^T and denominators (ones-vector matmul) accumulated
    on the PE. Normalize after the k-loop: DVE reciprocal straight from
    PSUM, GpSimd partition-broadcast, DVE multiply into bf16.
  - out rows = sum_h (av_h/den_h)^T @ Wo_h.

RoPE uses a host-side column permutation of Wq/Wk so the rotation partner of
partition p lives 16 partitions away within the same 32-partition quadrant
(one DVE stream_shuffle); cos/sin tables host-expanded to [128, T], signs
folded in. Softmax needs no max-subtraction: logits are O(1) by construction
(inputs are N(0,1)-scaled), max logit ~6.

All matmul inputs bf16 (fp32 accumulation); ~0.5% rel error vs fp32 ref.
"""

import math

import ml_dtypes
import numpy as np

import concourse.bass as bass
import concourse.mybir as mybir
import concourse.tile as tile
from concourse import bacc
from concourse.bass_utils import run_bass_kernel_spmd

B, T, D = 2, 4096, 896
H, HD = 7, 128
DC = D // 128  # 7 contraction chunks
NCORES = 8
QLOC = 1024  # local query rows per core
QW = 512  # query window width
NW = QLOC // QW  # 2
NKT = T // 128  # 32
MW = 32  # causal boundary mask width
SCALE = 1.0 / math.sqrt(HD)
LAG = 2  # software pipeline depth (chunks) between scores/exp and av/den

BF = mybir.dt.bfloat16
F32 = mybir.dt.float32
npbf = ml_dtypes.bfloat16

SHUF = [(i + 16) % 32 for i in range(32)]
EXP = mybir.ActivationFunctionType.Exp
PAIRS = [(0, 1), (2, 3), (4, 5), (6,)]


def _emit(nc, repeat=1):
    xt = nc.dram_tensor("xt", [D, T], BF, kind="ExternalInput").ap()
    xq = nc.dram_tensor("xq", [D, QLOC], BF, kind="ExternalInput").ap()
    wq = nc.dram_tensor("wq", [D, H * HD], BF, kind="ExternalInput").ap()
    wk = nc.dram_tensor("wk", [D, HD], BF, kind="ExternalInput").ap()
    wv = nc.dram_tensor("wv", [D, HD], BF, kind="ExternalInput").ap()
    wo = nc.dram_tensor("wo", [H * HD, D], BF, kind="ExternalInput").ap()
    cq = nc.dram_tensor("cq", [HD, QLOC], F32, kind="ExternalInput").ap()
    sq = nc.dram_tensor("sq", [HD, QLOC], F32, kind="ExternalInput").ap()
    ck = nc.dram_tensor("ck", [HD, T], F32, kind="ExternalInput").ap()
    sk = nc.dram_tensor("sk", [HD, T], F32, kind="ExternalInput").ap()
    msk = nc.dram_tensor("msk", [HD, 2 * MW], BF, kind="ExternalInput").ap()
    out = nc.dram_tensor("out", [QLOC, D], F32, kind="ExternalOutput").ap()

    xt_r = xt.rearrange("(dc p) t -> p dc t", p=128)
    xq_r = xq.rearrange("(dc p) t -> p dc t", p=128)

    with tile.TileContext(nc) as tc:
        with (
            tc.tile_pool(name="const", bufs=1) as const,
            tc.tile_pool(name="persist", bufs=1) as persist,
        ):
            # constants needed earliest first (k projection + rope)
            wk_sb = const.tile([128, DC, HD], BF)
            nc.sync.dma_start(out=wk_sb, in_=wk.rearrange("(dc p) n -> p dc n", p=128))
            wv_sb = const.tile([128, DC, HD], BF)
            nc.sync.dma_start(out=wv_sb, in_=wv.rearrange("(dc p) n -> p dc n", p=128))
            ck_sb = const.tile([128, T], F32)
            sk_sb = const.tile([128, T], F32)
            msk_sb = const.tile([128, 2, MW], BF)
            cq_sb = const.tile([128, QLOC], F32)
            sq_sb = const.tile([128, QLOC], F32)
            wq_sb = const.tile([128, DC, H * HD], BF)
            wo_sb = const.tile([128, H, D], BF)
            ones_sb = const.tile([128, 1], BF)
            nc.vector.memset(ones_sb, 1.0)

            for _rep in range(repeat):
                kr_sb = persist.tile([128, T], BF, tag="kr", name="kr")
                v_sb = persist.tile([128, NKT, 128], BF, tag="v", name="v")
                qr_sb = persist.tile([128, H, QLOC], BF, tag="qr", name="qr")

                with (
                    tc.tile_pool(name="xqpool", bufs=1) as xqpool,
                    tc.tile_pool(name="rt", bufs=3) as rt,
                ):
                    xq_sb = xqpool.tile([128, DC, QLOC], BF)

                    def rope(ps, cos_ap, sin_ap, out_ap):
                        qs = rt.tile([128, QW], F32, tag="qs", name="qs")
                        nc.vector.stream_shuffle(qs, ps, SHUF)
                        t1 = rt.tile([128, QW], F32, tag="t1", name="t1")
                        nc.vector.tensor_mul(t1, ps, cos_ap)
                        t2 = rt.tile([128, QW], F32, tag="t2", name="t2")
                        nc.vector.tensor_mul(t2, qs, sin_ap)
                        nc.vector.tensor_add(out_ap, t1, t2)

                    # ===== phase B: k/v projections + rope, xt streamed =====
                    with (
                        tc.tile_pool(name="xtw", bufs=3) as xtw,
                        tc.tile_pool(name="vtp", bufs=1) as vtp,
                        tc.tile_pool(name="pbk", bufs=3, space="PSUM") as pbk,
                    ):
                        vt_sb = vtp.tile([128, T], BF)
                        for w in range(T // QW):
                            sl = bass.ts(w, QW)
                            xw = xtw.tile([128, DC, QW], BF, tag="xw", name="xw")
                            nc.sync.dma_start(out=xw, in_=xt_r[:, :, sl])
                            nc.scalar.dma_start(out=ck_sb[:, sl], in_=ck[:, sl])
                            nc.scalar.dma_start(out=sk_sb[:, sl], in_=sk[:, sl])
                            if w == 2:
                                nc.sync.dma_start(out=xq_sb, in_=xq_r)
                            if w == 4:
                                nc.scalar.dma_start(out=cq_sb, in_=cq)
                                nc.scalar.dma_start(out=sq_sb, in_=sq)
                            if w == 5:
                                nc.scalar.dma_start(
                                    out=wq_sb,
                                    in_=wq.rearrange("(dc p) n -> p dc n", p=128),
                                )
                            ps = pbk.tile([128, QW], F32, tag="ps", name="ps")
                            for dc in range(DC):
                                nc.tensor.matmul(
                                    ps,
                                    lhsT=wk_sb[:, dc, :],
                                    rhs=xw[:, dc, :],
                                    start=dc == 0,
                                    stop=dc == DC - 1,
                                )
                            rope(ps, ck_sb[:, sl], sk_sb[:, sl], kr_sb[:, sl])
                            ps2 = pbk.tile([128, QW], F32, tag="ps", name="ps2")
                            for dc in range(DC):
                                nc.tensor.matmul(
                                    ps2,
                                    lhsT=wv_sb[:, dc, :],
                                    rhs=xw[:, dc, :],
                                    start=dc == 0,
                                    stop=dc == DC - 1,
                                )
                            nc.scalar.copy(vt_sb[:, sl], ps2)
                            nc.scalar.dma_start_transpose(
                                out=v_sb[:, w * (QW // 128) : (w + 1) * (QW // 128), :],
                                in_=vt_sb[:, sl],
                            )
                            if w >= 6:
                                h0 = w - 6  # heads 0,1 of pair 0
                                for wq_w in range(NW):
                                    qsl = bass.ts(wq_w, QW)
                                    qps = pbk.tile(
                                        [128, QW], F32, tag="ps", name="qps"
                                    )
                                    for dc in range(DC):
                                        nc.tensor.matmul(
                                            qps,
                                            lhsT=wq_sb[:, dc, bass.ts(h0, HD)],
                                            rhs=xq_sb[:, dc, qsl],
                                            start=dc == 0,
                                            stop=dc == DC - 1,
                                        )
                                    rope(
                                        qps, cq_sb[:, qsl], sq_sb[:, qsl],
                                        qr_sb[:, h0, qsl],
                                    )

                    # deferred constant loads
                    nc.scalar.dma_start(
                        out=msk_sb, in_=msk.rearrange("p (i w) -> p i w", i=2)
                    )
                    nc.scalar.dma_start(
                        out=wo_sb, in_=wo.rearrange("(h p) n -> p h n", p=128)
                    )

                    # ============ attention, per head pair ============
                    with (
                        tc.tile_pool(name="avpool", bufs=1) as avpool,
                        tc.tile_pool(name="expool", bufs=4) as expool,
                        tc.tile_pool(name="nrm", bufs=2) as nrm,
                        tc.tile_pool(name="outp", bufs=2) as outp,
                        tc.tile_pool(name="ps_sc", bufs=2, space="PSUM") as ps_sc,
                        tc.tile_pool(name="ps_av", bufs=2, space="PSUM") as ps_av,
                        tc.tile_pool(name="ps_dn", bufs=2, space="PSUM") as ps_dn,
                    ):
                        av_sb = avpool.tile([128, H, QLOC], BF)

                        def qproj(h):
                            # psum slots shared with denominators via tag
                            for w in range(NW):
                                sl = bass.ts(w, QW)
                                ps = ps_dn.tile([128, QW], F32, tag="dn", name="qps")
                                for dc in range(DC):
                                    nc.tensor.matmul(
                                        ps,
                                        lhsT=wq_sb[:, dc, bass.ts(h, HD)],
                                        rhs=xq_sb[:, dc, sl],
                                        start=dc == 0,
                                        stop=dc == DC - 1,
                                    )
                                rope(ps, cq_sb[:, sl], sq_sb[:, sl], qr_sb[:, h, sl])

                        for pi, hp in enumerate(PAIRS):
                            nh = len(hp)
                            for w in range(NW):
                                if w == 1 and pi + 1 < len(PAIRS):
                                    for h in PAIRS[pi + 1]:
                                        qproj(h)
                                n_k = 16 * (w + 1)
                                nd = n_k - 16
                                avp = [
                                    ps_av.tile([128, QW], F32, tag="av", name="avp")
                                    for _ in range(nh)
                                ]
                                dn_ps = ps_dn.tile([128, QW], F32, tag="dn", name="dnp")
                                dnp = [dn_ps[32 * i : 32 * i + 1, :] for i in range(nh)]

                                pend = []  # (ex, j0, kb)

                                def flush(last):
                                    ex, j0, kb = pend.pop(0)
                                    for i in range(nh):
                                        nc.tensor.matmul(
                                            avp[i][:, j0:],
                                            lhsT=v_sb[:, kb, :],
                                            rhs=ex[:, i, j0:],
                                            start=kb == 0,
                                            stop=last,
                                        )
                                    for i in range(nh):
                                        nc.tensor.matmul(
                                            dnp[i][:, j0:],
                                            lhsT=ones_sb,
                                            rhs=ex[:, i, j0:],
                                            start=kb == 0,
                                            stop=last,
                                        )

                                for kb in range(n_k):
                                    c = kb - nd
                                    j0 = 0 if c < 0 else 32 * c
                                    sc_ps = ps_sc.tile(
                                        [128, nh, QW], F32, tag="sc", name="sc"
                                    )
                                    for i, h in enumerate(hp):
                                        nc.tensor.matmul(
                                            sc_ps[:, i, j0:],
                                            lhsT=kr_sb[:, bass.ts(kb, 128)],
                                            rhs=qr_sb[:, h, w * QW + j0 : (w + 1) * QW],
                                            start=True,
                                            stop=True,
                                        )
                                    ex = expool.tile([128, nh, QW], BF, tag="ex", name="ex")
                                    nc.scalar.activation(
                                        ex[:, :, j0:], sc_ps[:, :, j0:], func=EXP,
                                        scale=SCALE,
                                    )
                                    if c >= 0:
                                        nc.vector.tensor_mul(
                                            ex[:, :, j0 : j0 + MW],
                                            ex[:, :, j0 : j0 + MW],
                                            msk_sb[:, :nh, :],
                                        )
                                    pend.append((ex, j0, kb))
                                    if len(pend) > LAG:
                                        flush(False)
                                while pend:
                                    flush(len(pend) == 1)

                                # normalize
                                for i, h in enumerate(hp):
                                    rec = nrm.tile([1, QW], F32, tag="rec", name="rec")
                                    nc.vector.reciprocal(rec, dnp[i])
                                    db = nrm.tile([128, QW], F32, tag="db", name="db")
                                    nc.gpsimd.partition_broadcast(db, rec)
                                    nc.vector.tensor_mul(
                                        av_sb[:, h, bass.ts(w, QW)], avp[i], db
                                    )

                        # ===== phase D: output projection (reuses sc slots) =====
                        NO = 448
                        for s in range(QLOC // 128):
                            ob = outp.tile([128, D], F32, tag="ob", name="ob")
                            for half in range(2):
                                o_ps = ps_sc.tile([128, NO], F32, tag="sc", name="o")
                                for h in range(H):
                                    nc.tensor.matmul(
                                        o_ps,
                                        lhsT=av_sb[:, h, bass.ts(s, 128)],
                                        rhs=wo_sb[:, h, bass.ts(half, NO)],
                                        start=h == 0,
                                        stop=h == H - 1,
                                    )
                                nc.vector.tensor_copy(ob[:, bass.ts(half, NO)], o_ps)
                            nc.sync.dma_start(out=out[bass.ts(s, 128), :], in_=ob)


_CACHE = {}


def _get_nc():
    if "nc" not in _CACHE:
        nc = bacc.Bacc("TRN2", target_bir_lowering=False, debug=False)
        _emit(nc)
        nc.compile()
        _CACHE["nc"] = nc
    return _CACHE["nc"]


def _rope_layout():
    p = np.arange(128)
    q, r = p // 32, p % 32
    j = 16 * q + (r % 16)
    is_b = r >= 16
    pi = 2 * j + is_b.astype(int)
    sign = np.where(is_b, 1.0, -1.0).astype(np.float32)
    return pi, j, sign


def _prepare_in_maps(x, Wq, Wk, Wv, Wo, cos, sin):
    x = np.asarray(x, dtype=np.float32)
    Wq = np.asarray(Wq, dtype=np.float32)
    Wk = np.asarray(Wk, dtype=np.float32)
    Wv = np.asarray(Wv, dtype=np.float32)
    Wo = np.asarray(Wo, dtype=np.float32)
    cos = np.asarray(cos, dtype=np.float32)
    sin = np.asarray(sin, dtype=np.float32)

    pi, j_of_p, sign = _rope_layout()
    wq_p = np.ascontiguousarray(
        Wq.reshape(D, H, HD)[:, :, pi].reshape(D, H * HD)
    ).astype(npbf)
    wk_p = np.ascontiguousarray(Wk[:, pi]).astype(npbf)
    wv_b = Wv.astype(npbf)
    wo_b = Wo.astype(npbf)
    cosE = np.ascontiguousarray(cos.T[j_of_p, :])  # [128, T] f32
    sinE = np.ascontiguousarray(sin.T[j_of_p, :] * sign[:, None])

    in_maps = []
    for c_id in range(NCORES):
        b, p = divmod(c_id, 4)
        rows = 4 * np.arange(QLOC) + p
        xt_b = np.ascontiguousarray(x[b].T).astype(npbf)  # [D, T]
        xq_b = np.ascontiguousarray(xt_b[:, rows])
        m1 = (np.arange(128)[:, None] <= 4 * np.arange(MW)[None, :] + p).astype(npbf)
        msk2 = np.concatenate([m1, m1], axis=1)  # duplicated for head pairs
        in_maps.append(
            {
                "xt": xt_b,
                "xq": xq_b,
                "wq": wq_p,
                "wk": wk_p,
                "wv": wv_b,
                "wo": wo_b,
                "cq": np.ascontiguousarray(cosE[:, rows]),
                "sq": np.ascontiguousarray(sinE[:, rows]),
                "ck": cosE,
                "sk": sinE,
                "msk": msk2,
            }
        )
    return in_maps


def kernel(x, Wq, Wk, Wv, Wo, cos, sin):
    in_maps = _prepare_in_maps(x, Wq, Wk, Wv, Wo, cos, sin)
    nc = _get_nc()
    res = run_bass_kernel_spmd(nc, in_maps, core_ids=list(range(NCORES)))
    out_full = np.zeros((B, T, D), dtype=np.float32)
    for c_id in range(NCORES):
        b, p = divmod(c_id, 4)
        out_full[b, p::4, :] = res.results[c_id]["out"]
    return out_full
